# revision 62
# baseline (speedup 1.0000x reference)
"""BERT attention + residual LayerNorm on 8 Trainium2 NeuronCores.

Sharding: data-parallel over batch (B=8 == n_cores), no collectives.
~220us/iter (vs 447us naive, 262us previous); rel err ~3.2e-3 (gate 2e-2).

Measured facts this design rests on (1000-iteration loop differencing;
the old docstring's numbers were artifacts of a noisy 100-iter bench):
  - any N=512 matmul costs ~270-280ns in a pipelined stream REGARDLESS
    of a fresh LDWEIGHTS (the PE pulls weight loads ahead into the
    background buffer); DoubleRow contracts 2x for the same cost.
  - K=64 matmuls in alternating row halves: ~175ns each.
  - K=1 broadcast matmuls are NOT cheap: ~410ns.
  - exp ACT FD=1024 ~1.2us, bias operand free -> the 128 softmax exps
    (~154us serialized) are the kernel floor: the scalar engine must
    do NOTHING else and never starve.
  - a 64<->128 row-mode switch costs ~66ns of drain.

Design:
  - fp8-e4m3 everywhere on the PE; all projections DoubleRow.
  - V layout [P,SC,NH,128]: head h's values in cols 64*(h%2)..+64, a
    1.0 column at col 96 (even h) / 32 (odd h), zeros elsewhere.  Each
    head's ctx DoubleRow chain then lands context at PSUM partitions
    64*(h%2)..+64 AND the softmax denominator r = sum_k E at row 96/32
    of its own bank -- the old separate r-matmul chain is gone.
  - attention mask rides in the exp ACT's per-partition bias operand
    (keys are partitions), so V needs no exp(mask) scaling.
  - ALL evacuations (Q/K bias, V bias scatter, out+residual) run on
    the DVE, LayerNorm's rsqrt(var+eps) is a DVE quake-bit-hack with
    two Newton steps (~1e-5): zero non-exp work on the scalar engine.
  - each tail's two ctx chains evacuate psum->SBUF (bf16) immediately
    (one tensor_copy), freeing the 2-slot ps_c ring after ~0.3us; the
    normalize (K=1 bcasts of the r rows -> recip -> mul into CT) runs
    entirely from SBUF.  Without this, tails serialize at the ~4.6us
    chain->bcast->recip->mul round-trip (+73us total, measured).
  - software pipeline: tail units of iteration i interleave into the
    score window of iteration i+2 (5 small units so no PE lump ever
    overruns the 2-bank pse ping-pong); V nt-halves and Q/K/out-proj
    chunk halves fill the remaining window slack.
  - gpsimd partition_broadcast gives wrong results on HW (sim agrees,
    HW does not) -- do not use it for the r broadcast.

PSUM banks (8): ps_e 2x[P,2,FS] (4) + ps_c 2x[P,FS] (2) +
ps_m 1x[P,FS] (1, filler chunks) + ps_s 1x[P,FS] (1, r broadcast).

Every dma_start destination is a virgin SBUF slot (DMA pseudo-instruction
has a single sync-wait slot; recycled destinations fail walrus codegen).
"""

from contextlib import ExitStack

import numpy as np
import ml_dtypes

import concourse.bass as bass
import concourse.bacc as bacc
import concourse.tile as tile
import concourse.mybir as mybir

BF16NP = ml_dtypes.bfloat16
F8NP = ml_dtypes.float8_e4m3
F32 = mybir.dt.float32
F16 = mybir.dt.float16
BF = mybir.dt.bfloat16
F8 = mybir.dt.float8e4
AF = mybir.ActivationFunctionType
ALU = mybir.AluOpType
DR = mybir.MatmulPerfMode.DoubleRow
I32 = mybir.dt.int32

P = 128

B, S, H, NH = 8, 1024, 1024, 16
LN_EPS = 1e-12
N_CORES = 8


def build_program(S=S, H=H, NH=NH, ln_eps=LN_EPS, n_cores=N_CORES, e_bufs=4,
                  loop_n=1, debug_taps=False, exp_fd2=True, ablate=None,
                  tail_mode="full"):
    timed = loop_n > 1
    SC, OC = S // P, H // P
    FS = min(512, S)
    NTS = S // FS
    FH = min(512, H)
    NTH = H // FH
    HD = H // NH
    HPC = P // HD          # heads per 128-row chunk (2)
    NPAIR = NH // HPC      # head-pair count (8)

    nc = bacc.Bacc(
        "TRN2",
        target_bir_lowering=False,
        debug=False,
        enable_asserts=False,
        num_devices=n_cores,
    )
    dt = nc.dram_tensor
    xt = dt("xt", [H, S], F8, kind="ExternalInput").ap()
    xb = dt("xb", [S, H], F16, kind="ExternalInput").ap()
    wq = dt("wq", [H, H], F8, kind="ExternalInput").ap()
    wk = dt("wk", [H, H], F8, kind="ExternalInput").ap()
    wv = dt("wv", [H, H], F8, kind="ExternalInput").ap()
    wo = dt("wo", [H, H], F8, kind="ExternalInput").ap()
    bqk = dt("bqk", [P, 2 * OC], F32, kind="ExternalInput").ap()  # bq*si | bk cols
    bvr = dt("bvr", [1, H], F16, kind="ExternalInput").ap()
    msk = dt("msk", [P, SC], F32, kind="ExternalInput").ap()
    gam = dt("gam", [1, H], F16, kind="ExternalInput").ap()
    bet = dt("bet", [1, H], F16, kind="ExternalInput").ap()
    rsc = dt("rsc", [2, FS], F32, kind="Internal").ap()  # r-row DMA bounce
    out = dt("out", [S, H], F16, kind="ExternalOutput").ap()
    if debug_taps:
        qt_o = dt("qt_o", [P, OC * S], F8, kind="ExternalOutput").ap()
        kt_o = dt("kt_o", [P, OC * S], F8, kind="ExternalOutput").ap()
        v_o = dt("v_o", [P, SC * NH * P], F8, kind="ExternalOutput").ap()
        e_o = dt("e_o", [P, SC * FS], F8, kind="ExternalOutput").ap()
        ct_o = dt("ct_o", [P, NPAIR * S], F8, kind="ExternalOutput").ap()

    with tile.TileContext(nc) as tc, ExitStack() as ctx:
        const = ctx.enter_context(tc.tile_pool(name="const", bufs=1))
        epool = ctx.enter_context(tc.tile_pool(name="epool", bufs=4))
        ctxpool = ctx.enter_context(tc.tile_pool(name="ctxpool", bufs=6))
        rbpool = ctx.enter_context(tc.tile_pool(name="rbpool", bufs=4))
        rspool = ctx.enter_context(tc.tile_pool(name="rspool", bufs=2))
        lnpool = ctx.enter_context(tc.tile_pool(name="lnpool", bufs=4))
        # 3-deep exp ping-pong (6 banks): PE detours up to ~3.6us no longer
        # starve the exp stream.  The tail's two head-chains serialize
        # through ONE ps_c bank (the SBUF evac makes that hand-off ~0.3us),
        # and the r broadcast needs no psum at all (DVE stream_shuffle).
        ps_e = ctx.enter_context(tc.tile_pool(name="ps_e", bufs=3, space="PSUM"))
        ps_c = ctx.enter_context(tc.tile_pool(name="ps_c", bufs=1, space="PSUM"))
        ps_m = ctx.enter_context(tc.tile_pool(name="ps_m", bufs=1, space="PSUM"))

        # XT shares its slot with CT when untimed (XT dead before CT writes);
        # in timed loop mode XT is needed again next iteration -> CT owns.
        XT = const.tile([P, OC, S], F8, tag="xtct", name="XT")
        WQ = const.tile([P, OC, H], F8, tag="WQ", name="WQ")
        WK = const.tile([P, OC, H], F8, tag="WK", name="WK")
        WV = const.tile([P, OC, H], F8, tag="WV", name="WV")
        WO = const.tile([P, OC, H], F8, tag="WO", name="WO")
        dmae = [nc.sync, nc.scalar, nc.gpsimd]
        _di = [0]

        def ld(dst, src):
            dmae[_di[0] % len(dmae)].dma_start(dst, src)
            _di[0] += 1

        for c in range(OC):
            ld(XT[:, c, :], xt[c * P:(c + 1) * P, :])
            ld(WK[:, c, :], wk[c * P:(c + 1) * P, :])
        for c in range(OC):
            ld(WQ[:, c, :], wq[c * P:(c + 1) * P, :])
            ld(WV[:, c, :], wv[c * P:(c + 1) * P, :])
        XB = const.tile([P, SC, H], F16, tag="XB", name="XB")
        for c in range(OC):
            ld(WO[:, c, :], wo[c * P:(c + 1) * P, :])
            ld(XB[:, c, :], xb[c * P:(c + 1) * P, :])
        QT = const.tile([P, OC, S], F8, tag="QT", name="QT")
        KT = const.tile([P, OC, S], F8, tag="KT", name="KT")
        # V in fp8, padded per head to 128 cols: head h lives in cols
        # 64*(h%2)..+64, the other 64 cols stay zero.  A DoubleRow ctx MM with
        # this lhsT writes head h's context to PSUM partitions 64(h%2)..+64
        # without tile_position (zeros accumulate harmlessly elsewhere).
        # One padding column per head is set to 1.0: col 96 for even heads,
        # col 32 for odd heads.  The per-head ctx chain then deposits the
        # softmax denominator r = sum_k E[k,q] at PSUM row 96 (even) / 32
        # (odd) of its own bank -- no separate r matmul chain needed.  (The
        # attention mask rides in the exp bias, not in V.)
        V = const.tile([P, SC, NH, P], F8, tag="V", name="V")
        nc.vector.memset(V, 0.0)
        for h in range(NH):
            rcol = 96 if h % 2 == 0 else 32
            nc.vector.memset(V[:, :, h, rcol:rcol + 1], 1.0)
        bqk_sb = const.tile([P, 2 * OC], F32, tag="bqk_sb", name="bqk_sb")
        nc.sync.dma_start(bqk_sb, bqk)
        # bv broadcast to all partitions, viewed [P, NH, HD] for the V evac
        bv_sb = const.tile([P, NH, HD], F16, tag="bv_sb", name="bv_sb")
        nc.sync.dma_start(bv_sb, bvr.to_broadcast([P, H]).rearrange(
            "p (h d) -> p h d", d=HD))
        ones_bf = const.tile([P, P], BF, tag="ones_bf", name="ones_bf")
        nc.vector.memset(ones_bf, 1.0)
        # additive mask per key, chunk-major: bias operand of the exp ACT
        msk_sb = const.tile([P, SC], F32, tag="msk_sb", name="msk_sb")
        nc.sync.dma_start(msk_sb, msk)
        gam_sb = const.tile([P, H], F16, tag="gam_sb", name="gam_sb")
        nc.sync.dma_start(gam_sb, gam.to_broadcast([P, H]))
        bet_sb = const.tile([P, H], F16, tag="bet_sb", name="bet_sb")
        nc.sync.dma_start(bet_sb, bet.to_broadcast([P, H]))
        eps_sb = const.tile([P, 1], F32, tag="eps_sb", name="eps_sb")
        nc.vector.memset(eps_sb, ln_eps)

        def phases():
            mm = nc.tensor.matmul
            CT = const.tile([P, NPAIR, S], F8,
                            tag="ct_own" if timed else "xtct", name="CT")

            # ---------- filler chunk emitters (PE work + DVE evacs) ----------
            # Projections run fp8 DoubleRow: one MM contracts a PAIR of
            # 128-row chunks (lhsT [K,2,M], rhs [K,2,N] -> out [M,N] summed
            # over the pair), halving the accumulation-chain length.  The two
            # nt output tiles run sequentially through ONE ps_m bank (weight
            # reloads are free: LDWEIGHTS pulls ahead into the background
            # buffer).  Chunks are emitted in 4 pieces (piece p: nt=p//2,
            # k-half=p%2) to fit the PE stall windows between score groups.
            def _pair_chunk(key, piece, lhs_fn, rhs_fn, kend, evac_fn,
                            pool=None):
                gend = kend // 2
                gh = gend // 2
                for p in (range(4) if piece is None else [piece]):
                    nt, half = divmod(p, 2)
                    if half == 0:
                        pl = pool or ps_m
                        _half[key] = pl.tile([P, FS], F32,
                                             tag="psc" if pl is ps_c
                                             else "psm", name="pm")
                    ps = _half[key]
                    for g in range(half * gh, gh + half * gh):
                        mm(ps, lhs_fn(g), rhs_fn(g, nt), start=(g == 0),
                           stop=(g == gend - 1), perf_mode=DR)
                    if half == 1:
                        evac_fn(nt, _half.pop(key))

            def qtkt_chunk(which, oc, piece=None):
                # which: 0 -> Q, 1 -> K.  dest[:, oc, :] = W @ X^T + b (both nt)
                wsb, dest = ((WQ, QT), (WK, KT))[which]

                def evac(nt, ps):
                    # evacuate on the DVE: the scalar engine must stream
                    # softmax exps back-to-back (it is the kernel bottleneck)
                    bcol = bqk_sb[:, which * OC + oc:which * OC + oc + 1]
                    nc.vector.tensor_scalar_add(
                        dest[:, oc, nt * FS:(nt + 1) * FS], ps, bcol)

                _pair_chunk(("qtkt", which, oc), piece,
                            lambda g: wsb[:, 2 * g:2 * g + 2, oc * P:(oc + 1) * P],
                            lambda g, nt: XT[:, 2 * g:2 * g + 2,
                                             nt * FS:(nt + 1) * FS],
                            OC, evac)

            def v_chunk(sc, piece=None):
                # V[:, sc, h, pad] = X @ Wv^T + bv; DVE adds the bias while
                # scattering psum cols h*64-nt*512 into the padded per-head
                # slots (the mask rides in the exp bias, not in V)
                def evac(nt, ps):
                    h0 = nt * (NH // NTH)  # first head of this nt block
                    for par in range(2):   # even/odd heads of the block
                        src = ps.rearrange("p (h d) -> p h d", d=HD)[
                            :, par::2, :]
                        dst = V[:, sc, h0 + par:h0 + NH // NTH:2,
                                64 * par:64 * par + HD]
                        nc.vector.tensor_add(
                            dst, src, bv_sb[:, h0 + par:h0 + NH // NTH:2, :])

                _pair_chunk(("v", sc), piece,
                            lambda g: XT[:, 2 * g:2 * g + 2, sc * P:(sc + 1) * P],
                            lambda g, nt: WV[:, 2 * g:2 * g + 2,
                                             nt * FH:(nt + 1) * FH],
                            OC, evac)

            def out_chunk(sc, resid, piece=None, pool=None):
                def evac(nt, ps):
                    nc.vector.tensor_add(resid[:, nt * FH:(nt + 1) * FH],
                                         ps, XB[:, sc, nt * FH:(nt + 1) * FH])

                _pair_chunk(("out", sc), piece,
                            lambda g: CT[:, 2 * g:2 * g + 2, sc * P:(sc + 1) * P],
                            lambda g, nt: WO[:, 2 * g:2 * g + 2,
                                             nt * FH:(nt + 1) * FH],
                            NPAIR, evac, pool=pool)

            _half = {}

            def ln_chunk(sc, resid):
                stats = lnpool.tile([P, NTH, 6], F32, tag="stats", name="stats")
                for g in range(NTH):
                    nc.vector.bn_stats(stats[:, g, :], resid[:, g * FH:(g + 1) * FH])
                mv = lnpool.tile([P, 2], F32, tag="mv", name="mv")
                nc.vector.bn_aggr(mv, stats)
                # rsqrt(var+eps) entirely on the DVE (quake bit-hack + two
                # Newton steps, ~1e-5 rel): keeps the scalar engine free to
                # stream softmax exps back-to-back.
                ve = lnpool.tile([P, 1], F32, tag="ve", name="ve")
                nc.vector.tensor_scalar_add(ve, mv[:, 1:2], float(ln_eps))
                sh = lnpool.tile([P, 1], I32, tag="sh", name="sh")
                nc.vector.tensor_scalar(sh, ve.bitcast(I32), 1, None,
                                        op0=ALU.arith_shift_right)
                seed = lnpool.tile([P, 1], I32, tag="seed", name="seed")
                nc.vector.tensor_scalar(seed, sh, -1, 0x5F3759DF,
                                        op0=ALU.mult, op1=ALU.add)
                hh = lnpool.tile([P, 1], F32, tag="hh", name="hh")
                nc.vector.tensor_scalar(hh, ve, -0.5, None, op0=ALU.mult)
                y = seed.bitcast(F32)
                for it in range(2):
                    sq = lnpool.tile([P, 1], F32, tag=f"sq{it}", name="sq")
                    nc.vector.tensor_mul(sq, y, y)
                    corr = lnpool.tile([P, 1], F32, tag=f"co{it}", name="co")
                    nc.vector.tensor_scalar(corr, sq, hh, 1.5,
                                            op0=ALU.mult, op1=ALU.add)
                    yn = lnpool.tile([P, 1], F32, tag=f"yn{it}", name="yn")
                    nc.vector.tensor_mul(yn, y, corr)
                    y = yn
                nc.vector.tensor_scalar(resid, resid, mv[:, 0:1], y,
                                        op0=ALU.subtract, op1=ALU.mult)
                nc.vector.tensor_mul(resid, resid, gam_sb)
                nc.vector.tensor_add(resid, resid, bet_sb)
                nc.sync.dma_start(out[sc * P:(sc + 1) * P, :], resid)

            def out_block(sc, piece=None, pool=None):
                # pieces 0..3: projection; piece 4: LayerNorm + DMA out
                if piece in (None, 0):
                    _resid[sc] = rspool.tile([P, H], F16, tag="resid",
                                             name="resid")
                if piece is None or piece < 4:
                    out_chunk(sc, _resid[sc], piece=piece, pool=pool)
                if piece in (None, 4):
                    ln_chunk(sc, _resid.pop(sc))

            _resid = {}

            # ---------- phase-2 attention: scores half + consumer tail ----------
            # The tail of iteration i is emitted AFTER the scores of i+1 so
            # the ACT engine never starves behind ctx/r in the PE FIFO.
            def attn_scores(qt, pc, fillers):
                # E for both heads of the pair lives in one tile so each
                # per-kc FD=1024 ACT (2 PSUM banks) writes contiguously.
                E = epool.tile([P, SC, HPC, FS], F8, tag="E", name="E")
                fi = list(fillers)
                for kc in range(SC):
                    pse = ps_e.tile([P, HPC, FS], F32, tag="pse", name="pse")
                    for j in range(HPC):
                        hp = j * HD
                        mm(pse[:, j, :],
                           KT[hp:hp + HD, pc, kc * P:(kc + 1) * P],
                           QT[hp:hp + HD, pc, qt * FS:(qt + 1) * FS],
                           start=True, stop=True)
                    nc.scalar.activation(E[:, kc, :, :], pse, AF.Exp,
                                         bias=msk_sb[:, kc:kc + 1])
                    # pop this slot's share of fillers.  Keep all 8 insertion
                    # points: batching fillers into fewer, larger lumps (to
                    # save ~66ns/row-mode-switch) overruns the 2.4us pse
                    # ping-pong buffer and starves the exp stream (measured
                    # +44us at 4 insertion points).
                    k0 = (kc * len(fi)) // SC
                    k1 = ((kc + 1) * len(fi)) // SC
                    for f in fi[k0:k1]:
                        f()
                if debug_taps and qt == 0 and pc == 0:
                    nc.sync.dma_start(e_o.rearrange("p (a b) -> p a b", a=SC), E[:, :, 0, :])
                return E

            def make_tail_units(qt, pc, E):
                # The tail as 5 small filler units so its PE work slots
                # between score groups (a lump tail overruns the 2-bank pse
                # ping-pong and starves the exp stream).  One DoubleRow chain
                # per head, each into its own bank: head h's padded V slice
                # lands its context at PSUM partitions 64(h%2)..+64 and its
                # denominator r (via the 1.0 column) at row 96 (even head,
                # bank a) / row 32 (odd head, bank b).
                cols = slice(qt * FS, (qt + 1) * FS)
                st = {}

                def chain_piece(j, half):
                    def f():
                        if half == 0:
                            st[j] = ps_c.tile([P, FS], F32, tag="psc",
                                              name=f"pc{j}")
                        ps = st[j]
                        h = pc * HPC + j
                        for g in ((0, 1) if half == 0 else (2, 3)):
                            mm(ps, V[:, 2 * g:2 * g + 2, h, :],
                               E[:, 2 * g:2 * g + 2, j, :],
                               start=(g == 0), stop=(g == 3), perf_mode=DR)
                        if half == 1:
                            # evacuate ctx+r to SBUF (bf16) at once: the psum
                            # bank frees after ~0.3us instead of after the
                            # whole normalize round-trip, so successive tails
                            # don't serialize on the 2-bank ps_c ring.
                            cs = ctxpool.tile([P, FS], BF, tag="cs",
                                              name=f"cs{j}")
                            nc.vector.tensor_copy(cs, st.pop(j))
                            st[f"c{j}"] = cs
                    return f

                def norm():
                    ca, cb = st["c0"], st["c1"]
                    if tail_mode in ("ctx", "ctxr"):
                        nc.vector.tensor_copy(CT[0:64, pc, cols], ca[0:64, :])
                        nc.vector.tensor_copy(CT[64:P, pc, cols], cb[64:P, :])
                        return
                    # r broadcast with NO PE/PSUM involvement (proven on
                    # HW): stage r rows to f32, DMA to lane 0 of each
                    # 32-partition quadrant, one stream_shuffle replicates.
                    tr = rbpool.tile([P, FS], F32, tag="tr", name="tr")
                    nc.vector.tensor_copy(tr[96:97, :], ca[96:97, :])
                    nc.vector.tensor_copy(tr[32:33, :], cb[32:33, :])
                    rr = rbpool.tile([P, FS], F32, tag="rr", name="rr")
                    nc.gpsimd.dma_start(rr[0:1, :], tr[96:97, :])
                    nc.gpsimd.dma_start(rr[32:33, :], tr[96:97, :])
                    nc.gpsimd.dma_start(rr[64:65, :], tr[32:33, :])
                    nc.gpsimd.dma_start(rr[96:97, :], tr[32:33, :])
                    rs = rbpool.tile([P, FS], F32, tag="rs", name="rs")
                    nc.vector.stream_shuffle(rs, rr, mask=[0] * 32)
                    rb = rbpool.tile([P, FS], F32, tag="rb", name="rb")
                    nc.vector.reciprocal_approx_fast(out=rb, in_=rs)
                    nc.vector.tensor_mul(CT[0:HD, pc, cols], ca[0:HD, :],
                                         rb[0:HD, :])
                    nc.vector.tensor_mul(CT[HD:P, pc, cols], cb[HD:P, :],
                                         rb[HD:P, :])

                return [chain_piece(0, 0), chain_piece(0, 1),
                        chain_piece(1, 0), chain_piece(1, 1), norm]

            def attn_tail(qt, pc, E):
                for u in make_tail_units(qt, pc, E):
                    u()

            def attn_iter(qt, pc, fillers):
                E = attn_scores(qt, pc, fillers)
                attn_tail(qt, pc, E)

            # ---------- schedule ----------
            if ablate is not None:
                if "attn" not in ablate and "out" not in ablate:
                    nc.vector.memset(CT[:, 0, 0:16], 0.0)
                # partial builds for phase-cost ablation (timing only)
                if "qtkt" in ablate:
                    for c in range(OC):
                        for w in (1, 0):
                            qtkt_chunk(w, c)
                if "v" in ablate:
                    for sc in range(SC):
                        v_chunk(sc)
                if "sconly" in ablate:
                    for qt in range(NTS):
                        for pc in range(NPAIR):
                            attn_scores(qt, pc, [])
                elif "attnpipe" in ablate:
                    pend = None
                    for qt in range(NTS):
                        for pc in range(NPAIR):
                            E = attn_scores(qt, pc, [])
                            if pend is not None:
                                attn_tail(*pend)
                            pend = (qt, pc, E)
                    attn_tail(*pend)
                elif "attn" in ablate:
                    for qt in range(NTS):
                        for pc in range(NPAIR):
                            attn_iter(qt, pc, [])
                if "out" in ablate:
                    for sc in range(SC):
                        out_block(sc)
                return

            # prologue: QT/KT for pair 0
            qtkt_chunk(1, 0)
            qtkt_chunk(0, 0)

            def halves(f, *args):
                # two filler units per chunk; each unit emits one nt-half
                # (pieces 2p, 2p+1 adjacent: the ps_m slot is held between
                # them, so a unit is atomic wrt other ps_m chunks)
                def unit(p):
                    f(*args, piece=2 * p)
                    f(*args, piece=2 * p + 1)
                return [(lambda pp=pp: unit(pp)) for pp in range(2)]

            fill = {}
            for pc in range(NPAIR):
                fill[(0, pc)] = []
                fill[(1, pc)] = []
            # K(c) + Q(c) during iter (0, c-1).  (Moving the Q-nt1 halves to
            # their qt=1 consumer windows deadlocks the schedule -- the
            # extra ps_m users in qt1 windows close a cross-FIFO cycle with
            # the tail-unit ctx evacuations.)
            for c in range(1, OC):
                fill[(0, c - 1)] += (halves(qtkt_chunk, 1, c)
                                     + halves(qtkt_chunk, 0, c))
            # V nt0 (heads 0-7) must precede tail(0,0)'s units, interleaved
            # into window (0,2); V nt1 (heads 8-15) precedes tail(0,4) in
            # window (0,6).
            vh = {sc: halves(v_chunk, sc) for sc in range(SC)}
            for w, scs in ((0, (0, 1, 2, 3)), (1, (4, 5, 6, 7))):
                fill[(0, w)] += [vh[sc][0] for sc in scs]
            for w, scs in ((2, (0, 1)), (3, (2, 3)), (4, (4, 5)), (5, (6, 7))):
                fill[(0, w)] += [vh[sc][1] for sc in scs]
            # phase-3 for sc 0-3 (CT qt0 columns complete with tail (0,7)'s
            # units inside window (1,1) at pipeline depth 2)
            for i, sc in enumerate(range(4)):
                fill[(1, 2 + i)] += (halves(out_block, sc)
                                     + [lambda sc=sc: out_block(sc, piece=4)])

            # pipelined: tail units of iteration i interleave into the score
            # window of iteration i+2
            pending = []
            for qt in range(NTS):
                for pc in range(NPAIR):
                    tu = pending.pop(0) if len(pending) >= 2 else []
                    E = attn_scores(qt, pc, tu + fill[(qt, pc)])
                    pending.append(make_tail_units(qt, pc, E))
            for units in pending:
                for u in units:
                    u()

            # epilogue: remaining output blocks, alternating between the
            # ps_m and (now idle) ps_s banks so consecutive half-chunks
            # pipeline instead of serializing on one bank's evac round-trip
            for sc in range(4, SC):
                out_block(sc, pool=(ps_c if sc % 2 == 0 else ps_m))

            if debug_taps:
                nc.sync.dma_start(qt_o, QT.rearrange("p a b -> p (a b)"))
                nc.sync.dma_start(kt_o, KT.rearrange("p a b -> p (a b)"))
                nc.sync.dma_start(v_o.rearrange("p (a b c) -> p a b c", a=SC, b=NH), V)
                nc.sync.dma_start(ct_o, CT.rearrange("p a b -> p (a b)"))

        if loop_n == 1:
            phases()
        else:
            with tc.For_i(0, loop_n, 1) as _i:
                phases()
    nc.compile()
    return nc


def make_in_maps(input_tensor, attention_mask, Wq, bq, Wk, bk, Wv, bv, Wo, bo,
                 ln_gamma, ln_beta, S=S, H=H, NH=NH):
    SC = S // P
    OC = H // P
    X = np.asarray(input_tensor, np.float32)
    mask = np.asarray(attention_mask, np.float32)
    Wq = np.asarray(Wq, np.float32)
    Wk = np.asarray(Wk, np.float32)
    Wv = np.asarray(Wv, np.float32)
    Wo = np.asarray(Wo, np.float32)
    bq = np.asarray(bq, np.float32)
    bk = np.asarray(bk, np.float32)
    bv = np.asarray(bv, np.float32)
    bo = np.asarray(bo, np.float32)
    HD = H // NH
    sc_inv = 1.0 / np.sqrt(np.float32(HD))

    bqk = np.concatenate([
        (bq * sc_inv).reshape(OC, P).T,
        bk.reshape(OC, P).T,
    ], axis=1).astype(np.float32)  # [P, 2*OC]

    shared = {
        "wq": np.ascontiguousarray((Wq * sc_inv).T).astype(F8NP),
        "wk": np.ascontiguousarray(Wk.T).astype(F8NP),
        "wv": np.ascontiguousarray(Wv.T).astype(F8NP),
        "wo": np.ascontiguousarray(Wo.T).astype(F8NP),
        "bqk": np.ascontiguousarray(bqk),
        "bvr": bv[None, :].astype(np.float16),
        "gam": np.asarray(ln_gamma, np.float32)[None, :].astype(np.float16),
        "bet": np.asarray(ln_beta, np.float32)[None, :].astype(np.float16),
    }
    in_maps = []
    for b in range(X.shape[0]):
        m = dict(shared)
        m["xt"] = np.ascontiguousarray(X[b].T).astype(F8NP)
        m["xb"] = np.ascontiguousarray(X[b] + bo[None, :]).astype(np.float16)
        m["msk"] = np.ascontiguousarray(mask[b].reshape(SC, P).T.astype(np.float32))
        in_maps.append(m)
    return in_maps


_CACHE = {}


def _get_nc():
    if "nc" not in _CACHE:
        _CACHE["nc"] = build_program()
    return _CACHE["nc"]


def run_on_hw(in_maps, trace=False, **kw):
    from concourse.bass_utils import run_bass_kernel_spmd

    nc = _get_nc()
    return run_bass_kernel_spmd(nc, in_maps, list(range(N_CORES)), trace=trace, **kw)


def kernel(input_tensor, attention_mask, Wq, bq, Wk, bk, Wv, bv, Wo, bo,
           ln_gamma, ln_beta):
    in_maps = make_in_maps(input_tensor, attention_mask, Wq, bq, Wk, bk, Wv, bv,
                           Wo, bo, ln_gamma, ln_beta)
    res = run_on_hw(in_maps)
    return np.stack([res.results[b]["out"] for b in range(N_CORES)]).astype(np.float32)



# revision 63
# speedup vs baseline: 1.2110x; 1.2110x over previous
"""BERT attention + residual LayerNorm on 8 Trainium2 NeuronCores.

Sharding: data-parallel over batch (B=8 == n_cores), no collectives.
~220us/iter (vs 447us naive, 262us previous); rel err ~3.2e-3 (gate 2e-2).

Measured facts this design rests on (1000-iteration loop differencing;
the old docstring's numbers were artifacts of a noisy 100-iter bench):
  - any N=512 matmul costs ~270-280ns in a pipelined stream REGARDLESS
    of a fresh LDWEIGHTS (the PE pulls weight loads ahead into the
    background buffer); DoubleRow contracts 2x for the same cost.
  - K=64 matmuls in alternating row halves: ~175ns each.
  - K=1 broadcast matmuls are NOT cheap: ~410ns.
  - exp ACT FD=1024 ~1.2us, bias operand free -> the 128 softmax exps
    (~154us serialized) are the kernel floor: the scalar engine must
    do NOTHING else and never starve.
  - a 64<->128 row-mode switch costs ~66ns of drain.

Design:
  - fp8-e4m3 everywhere on the PE; all projections DoubleRow.
  - V layout [P,SC,NH,128]: head h's values in cols 64*(h%2)..+64, a
    1.0 column at col 96 (even h) / 32 (odd h), zeros elsewhere.  Each
    head's ctx DoubleRow chain then lands context at PSUM partitions
    64*(h%2)..+64 AND the softmax denominator r = sum_k E at row 96/32
    of its own bank -- the old separate r-matmul chain is gone.
  - attention mask rides in the exp ACT's per-partition bias operand
    (keys are partitions), so V needs no exp(mask) scaling.
  - ALL evacuations (Q/K bias, V bias scatter, out+residual) run on
    the DVE, LayerNorm's rsqrt(var+eps) is a DVE quake-bit-hack with
    two Newton steps (~1e-5): zero non-exp work on the scalar engine.
  - each tail's two ctx chains evacuate psum->SBUF (bf16) immediately
    (one tensor_copy), freeing the 2-slot ps_c ring after ~0.3us; the
    normalize (K=1 bcasts of the r rows -> recip -> mul into CT) runs
    entirely from SBUF.  Without this, tails serialize at the ~4.6us
    chain->bcast->recip->mul round-trip (+73us total, measured).
  - software pipeline: tail units of iteration i interleave into the
    score window of iteration i+2 (5 small units so no PE lump ever
    overruns the 2-bank pse ping-pong); V nt-halves and Q/K/out-proj
    chunk halves fill the remaining window slack.
  - gpsimd partition_broadcast gives wrong results on HW (sim agrees,
    HW does not) -- do not use it for the r broadcast.

PSUM banks (8): ps_e 2x[P,2,FS] (4) + ps_c 2x[P,FS] (2) +
ps_m 1x[P,FS] (1, filler chunks) + ps_s 1x[P,FS] (1, r broadcast).

Every dma_start destination is a virgin SBUF slot (DMA pseudo-instruction
has a single sync-wait slot; recycled destinations fail walrus codegen).
"""

from contextlib import ExitStack

import numpy as np
import ml_dtypes

import concourse.bass as bass
import concourse.bacc as bacc
import concourse.tile as tile
import concourse.mybir as mybir

BF16NP = ml_dtypes.bfloat16
F8NP = ml_dtypes.float8_e4m3
F32 = mybir.dt.float32
F16 = mybir.dt.float16
BF = mybir.dt.bfloat16
F8 = mybir.dt.float8e4
AF = mybir.ActivationFunctionType
ALU = mybir.AluOpType
DR = mybir.MatmulPerfMode.DoubleRow
I32 = mybir.dt.int32

P = 128

B, S, H, NH = 8, 1024, 1024, 16
LN_EPS = 1e-12
N_CORES = 8


def build_program(S=S, H=H, NH=NH, ln_eps=LN_EPS, n_cores=N_CORES, e_bufs=4,
                  loop_n=1, debug_taps=False, exp_fd2=True, ablate=None,
                  tail_mode="full"):
    timed = loop_n > 1
    SC, OC = S // P, H // P
    FS = min(512, S)
    NTS = S // FS
    FH = min(512, H)
    NTH = H // FH
    HD = H // NH
    HPC = P // HD          # heads per 128-row chunk (2)
    NPAIR = NH // HPC      # head-pair count (8)

    nc = bacc.Bacc(
        "TRN2",
        target_bir_lowering=False,
        debug=False,
        enable_asserts=False,
        num_devices=n_cores,
    )
    dt = nc.dram_tensor
    xt = dt("xt", [H, S], F8, kind="ExternalInput").ap()
    xb = dt("xb", [S, H], F16, kind="ExternalInput").ap()
    wq = dt("wq", [H, H], F8, kind="ExternalInput").ap()
    wk = dt("wk", [H, H], F8, kind="ExternalInput").ap()
    wv = dt("wv", [H, H], F8, kind="ExternalInput").ap()
    wo = dt("wo", [H, H], F8, kind="ExternalInput").ap()
    bqk = dt("bqk", [P, 2 * OC], F32, kind="ExternalInput").ap()  # bq*si | bk cols
    bvr = dt("bvr", [1, H], F16, kind="ExternalInput").ap()
    msk = dt("msk", [P, SC], F32, kind="ExternalInput").ap()
    gam = dt("gam", [1, H], F16, kind="ExternalInput").ap()
    bet = dt("bet", [1, H], F16, kind="ExternalInput").ap()
    rsc = dt("rsc", [2, FS], F32, kind="Internal").ap()  # r-row DMA bounce
    out = dt("out", [S, H], F16, kind="ExternalOutput").ap()
    if debug_taps:
        qt_o = dt("qt_o", [P, OC * S], F8, kind="ExternalOutput").ap()
        kt_o = dt("kt_o", [P, OC * S], F8, kind="ExternalOutput").ap()
        v_o = dt("v_o", [P, SC * NH * P], F8, kind="ExternalOutput").ap()
        e_o = dt("e_o", [P, SC * FS], F8, kind="ExternalOutput").ap()
        ct_o = dt("ct_o", [P, NPAIR * S], F8, kind="ExternalOutput").ap()

    with tile.TileContext(nc) as tc, ExitStack() as ctx:
        const = ctx.enter_context(tc.tile_pool(name="const", bufs=1))
        epool = ctx.enter_context(tc.tile_pool(name="epool", bufs=4))
        ctxpool = ctx.enter_context(tc.tile_pool(name="ctxpool", bufs=6))
        rbpool = ctx.enter_context(tc.tile_pool(name="rbpool", bufs=4))
        rspool = ctx.enter_context(tc.tile_pool(name="rspool", bufs=2))
        lnpool = ctx.enter_context(tc.tile_pool(name="lnpool", bufs=4))
        ps_e = ctx.enter_context(tc.tile_pool(name="ps_e", bufs=2, space="PSUM"))
        ps_c = ctx.enter_context(tc.tile_pool(name="ps_c", bufs=2, space="PSUM"))
        ps_m = ctx.enter_context(tc.tile_pool(name="ps_m", bufs=1, space="PSUM"))
        ps_s = ctx.enter_context(tc.tile_pool(name="ps_s", bufs=1, space="PSUM"))

        # XT shares its slot with CT when untimed (XT dead before CT writes);
        # in timed loop mode XT is needed again next iteration -> CT owns.
        XT = const.tile([P, OC, S], F8, tag="xtct", name="XT")
        WQ = const.tile([P, OC, H], F8, tag="WQ", name="WQ")
        WK = const.tile([P, OC, H], F8, tag="WK", name="WK")
        WV = const.tile([P, OC, H], F8, tag="WV", name="WV")
        WO = const.tile([P, OC, H], F8, tag="WO", name="WO")
        dmae = [nc.sync, nc.scalar, nc.gpsimd]
        _di = [0]

        def ld(dst, src):
            dmae[_di[0] % len(dmae)].dma_start(dst, src)
            _di[0] += 1

        for c in range(OC):
            ld(XT[:, c, :], xt[c * P:(c + 1) * P, :])
            ld(WK[:, c, :], wk[c * P:(c + 1) * P, :])
        for c in range(OC):
            ld(WQ[:, c, :], wq[c * P:(c + 1) * P, :])
            ld(WV[:, c, :], wv[c * P:(c + 1) * P, :])
        XB = const.tile([P, SC, H], F16, tag="XB", name="XB")
        for c in range(OC):
            ld(WO[:, c, :], wo[c * P:(c + 1) * P, :])
            ld(XB[:, c, :], xb[c * P:(c + 1) * P, :])
        QT = const.tile([P, OC, S], F8, tag="QT", name="QT")
        KT = const.tile([P, OC, S], F8, tag="KT", name="KT")
        # V in fp8, padded per head to 128 cols: head h lives in cols
        # 64*(h%2)..+64, the other 64 cols stay zero.  A DoubleRow ctx MM with
        # this lhsT writes head h's context to PSUM partitions 64(h%2)..+64
        # without tile_position (zeros accumulate harmlessly elsewhere).
        # One padding column per head is set to 1.0: col 96 for even heads,
        # col 32 for odd heads.  The per-head ctx chain then deposits the
        # softmax denominator r = sum_k E[k,q] at PSUM row 96 (even) / 32
        # (odd) of its own bank -- no separate r matmul chain needed.  (The
        # attention mask rides in the exp bias, not in V.)
        V = const.tile([P, SC, NH, P], F8, tag="V", name="V")
        nc.vector.memset(V, 0.0)
        for h in range(NH):
            rcol = 96 if h % 2 == 0 else 32
            nc.vector.memset(V[:, :, h, rcol:rcol + 1], 1.0)
        bqk_sb = const.tile([P, 2 * OC], F32, tag="bqk_sb", name="bqk_sb")
        nc.sync.dma_start(bqk_sb, bqk)
        # bv broadcast to all partitions, viewed [P, NH, HD] for the V evac
        bv_sb = const.tile([P, NH, HD], F16, tag="bv_sb", name="bv_sb")
        nc.sync.dma_start(bv_sb, bvr.to_broadcast([P, H]).rearrange(
            "p (h d) -> p h d", d=HD))
        ones_bf = const.tile([P, P], BF, tag="ones_bf", name="ones_bf")
        nc.vector.memset(ones_bf, 1.0)
        # additive mask per key, chunk-major: bias operand of the exp ACT
        msk_sb = const.tile([P, SC], F32, tag="msk_sb", name="msk_sb")
        nc.sync.dma_start(msk_sb, msk)
        gam_sb = const.tile([P, H], F16, tag="gam_sb", name="gam_sb")
        nc.sync.dma_start(gam_sb, gam.to_broadcast([P, H]))
        bet_sb = const.tile([P, H], F16, tag="bet_sb", name="bet_sb")
        nc.sync.dma_start(bet_sb, bet.to_broadcast([P, H]))
        eps_sb = const.tile([P, 1], F32, tag="eps_sb", name="eps_sb")
        nc.vector.memset(eps_sb, ln_eps)

        def phases():
            mm = nc.tensor.matmul
            CT = const.tile([P, NPAIR, S], F8,
                            tag="ct_own" if timed else "xtct", name="CT")

            # ---------- filler chunk emitters (PE work + DVE evacs) ----------
            # Projections run fp8 DoubleRow: one MM contracts a PAIR of
            # 128-row chunks (lhsT [K,2,M], rhs [K,2,N] -> out [M,N] summed
            # over the pair), halving the accumulation-chain length.  The two
            # nt output tiles run sequentially through ONE ps_m bank (weight
            # reloads are free: LDWEIGHTS pulls ahead into the background
            # buffer).  Chunks are emitted in 4 pieces (piece p: nt=p//2,
            # k-half=p%2) to fit the PE stall windows between score groups.
            def _pair_chunk(key, piece, lhs_fn, rhs_fn, kend, evac_fn,
                            pool=None):
                gend = kend // 2
                gh = gend // 2
                for p in (range(4) if piece is None else [piece]):
                    nt, half = divmod(p, 2)
                    if half == 0:
                        pl = pool or ps_m
                        _half[key] = pl.tile([P, FS], F32,
                                             tag="pss" if pl is ps_s
                                             else "psm", name="pm")
                    ps = _half[key]
                    for g in range(half * gh, gh + half * gh):
                        mm(ps, lhs_fn(g), rhs_fn(g, nt), start=(g == 0),
                           stop=(g == gend - 1), perf_mode=DR)
                    if half == 1:
                        evac_fn(nt, _half.pop(key))

            def qtkt_chunk(which, oc, piece=None):
                # which: 0 -> Q, 1 -> K.  dest[:, oc, :] = W @ X^T + b (both nt)
                wsb, dest = ((WQ, QT), (WK, KT))[which]

                def evac(nt, ps):
                    # evacuate on the DVE: the scalar engine must stream
                    # softmax exps back-to-back (it is the kernel bottleneck)
                    bcol = bqk_sb[:, which * OC + oc:which * OC + oc + 1]
                    nc.vector.tensor_scalar_add(
                        dest[:, oc, nt * FS:(nt + 1) * FS], ps, bcol)

                _pair_chunk(("qtkt", which, oc), piece,
                            lambda g: wsb[:, 2 * g:2 * g + 2, oc * P:(oc + 1) * P],
                            lambda g, nt: XT[:, 2 * g:2 * g + 2,
                                             nt * FS:(nt + 1) * FS],
                            OC, evac)

            def v_chunk(sc, piece=None):
                # V[:, sc, h, pad] = X @ Wv^T + bv; DVE adds the bias while
                # scattering psum cols h*64-nt*512 into the padded per-head
                # slots (the mask rides in the exp bias, not in V)
                def evac(nt, ps):
                    h0 = nt * (NH // NTH)  # first head of this nt block
                    for par in range(2):   # even/odd heads of the block
                        src = ps.rearrange("p (h d) -> p h d", d=HD)[
                            :, par::2, :]
                        dst = V[:, sc, h0 + par:h0 + NH // NTH:2,
                                64 * par:64 * par + HD]
                        nc.vector.tensor_add(
                            dst, src, bv_sb[:, h0 + par:h0 + NH // NTH:2, :])

                _pair_chunk(("v", sc), piece,
                            lambda g: XT[:, 2 * g:2 * g + 2, sc * P:(sc + 1) * P],
                            lambda g, nt: WV[:, 2 * g:2 * g + 2,
                                             nt * FH:(nt + 1) * FH],
                            OC, evac)

            def out_chunk(sc, resid, piece=None, pool=None):
                def evac(nt, ps):
                    nc.vector.tensor_add(resid[:, nt * FH:(nt + 1) * FH],
                                         ps, XB[:, sc, nt * FH:(nt + 1) * FH])

                _pair_chunk(("out", sc), piece,
                            lambda g: CT[:, 2 * g:2 * g + 2, sc * P:(sc + 1) * P],
                            lambda g, nt: WO[:, 2 * g:2 * g + 2,
                                             nt * FH:(nt + 1) * FH],
                            NPAIR, evac, pool=pool)

            _half = {}

            def ln_chunk(sc, resid):
                stats = lnpool.tile([P, NTH, 6], F32, tag="stats", name="stats")
                for g in range(NTH):
                    nc.vector.bn_stats(stats[:, g, :], resid[:, g * FH:(g + 1) * FH])
                mv = lnpool.tile([P, 2], F32, tag="mv", name="mv")
                nc.vector.bn_aggr(mv, stats)
                # rsqrt(var+eps) entirely on the DVE (quake bit-hack + two
                # Newton steps, ~1e-5 rel): keeps the scalar engine free to
                # stream softmax exps back-to-back.
                ve = lnpool.tile([P, 1], F32, tag="ve", name="ve")
                nc.vector.tensor_scalar_add(ve, mv[:, 1:2], float(ln_eps))
                sh = lnpool.tile([P, 1], I32, tag="sh", name="sh")
                nc.vector.tensor_scalar(sh, ve.bitcast(I32), 1, None,
                                        op0=ALU.arith_shift_right)
                seed = lnpool.tile([P, 1], I32, tag="seed", name="seed")
                nc.vector.tensor_scalar(seed, sh, -1, 0x5F3759DF,
                                        op0=ALU.mult, op1=ALU.add)
                hh = lnpool.tile([P, 1], F32, tag="hh", name="hh")
                nc.vector.tensor_scalar(hh, ve, -0.5, None, op0=ALU.mult)
                y = seed.bitcast(F32)
                for it in range(2):
                    sq = lnpool.tile([P, 1], F32, tag=f"sq{it}", name="sq")
                    nc.vector.tensor_mul(sq, y, y)
                    corr = lnpool.tile([P, 1], F32, tag=f"co{it}", name="co")
                    nc.vector.tensor_scalar(corr, sq, hh, 1.5,
                                            op0=ALU.mult, op1=ALU.add)
                    yn = lnpool.tile([P, 1], F32, tag=f"yn{it}", name="yn")
                    nc.vector.tensor_mul(yn, y, corr)
                    y = yn
                nc.vector.tensor_scalar(resid, resid, mv[:, 0:1], y,
                                        op0=ALU.subtract, op1=ALU.mult)
                nc.vector.tensor_mul(resid, resid, gam_sb)
                nc.vector.tensor_add(resid, resid, bet_sb)
                nc.sync.dma_start(out[sc * P:(sc + 1) * P, :], resid)

            def out_block(sc, piece=None, pool=None):
                # pieces 0..3: projection; piece 4: LayerNorm + DMA out
                if piece in (None, 0):
                    _resid[sc] = rspool.tile([P, H], F16, tag="resid",
                                             name="resid")
                if piece is None or piece < 4:
                    out_chunk(sc, _resid[sc], piece=piece, pool=pool)
                if piece in (None, 4):
                    ln_chunk(sc, _resid.pop(sc))

            _resid = {}

            # ---------- phase-2 attention: scores half + consumer tail ----------
            # The tail of iteration i is emitted AFTER the scores of i+1 so
            # the ACT engine never starves behind ctx/r in the PE FIFO.
            def attn_scores(qt, pc, fillers):
                # E for both heads of the pair lives in one tile so each
                # per-kc FD=1024 ACT (2 PSUM banks) writes contiguously.
                E = epool.tile([P, SC, HPC, FS], F8, tag="E", name="E")
                fi = list(fillers)
                for kc in range(SC):
                    pse = ps_e.tile([P, HPC, FS], F32, tag="pse", name="pse")
                    for j in range(HPC):
                        hp = j * HD
                        mm(pse[:, j, :],
                           KT[hp:hp + HD, pc, kc * P:(kc + 1) * P],
                           QT[hp:hp + HD, pc, qt * FS:(qt + 1) * FS],
                           start=True, stop=True)
                    nc.scalar.activation(E[:, kc, :, :], pse, AF.Exp,
                                         bias=msk_sb[:, kc:kc + 1])
                    # pop this slot's share of fillers.  Keep all 8 insertion
                    # points: batching fillers into fewer, larger lumps (to
                    # save ~66ns/row-mode-switch) overruns the 2.4us pse
                    # ping-pong buffer and starves the exp stream (measured
                    # +44us at 4 insertion points).
                    k0 = (kc * len(fi)) // SC
                    k1 = ((kc + 1) * len(fi)) // SC
                    for f in fi[k0:k1]:
                        f()
                if debug_taps and qt == 0 and pc == 0:
                    nc.sync.dma_start(e_o.rearrange("p (a b) -> p a b", a=SC), E[:, :, 0, :])
                return E

            def make_tail_units(qt, pc, E):
                # The tail as 5 small filler units so its PE work slots
                # between score groups (a lump tail overruns the 2-bank pse
                # ping-pong and starves the exp stream).  One DoubleRow chain
                # per head, each into its own bank: head h's padded V slice
                # lands its context at PSUM partitions 64(h%2)..+64 and its
                # denominator r (via the 1.0 column) at row 96 (even head,
                # bank a) / row 32 (odd head, bank b).
                cols = slice(qt * FS, (qt + 1) * FS)
                st = {}

                def chain_piece(j, half):
                    def f():
                        if half == 0:
                            st[j] = ps_c.tile([P, FS], F32, tag="psc",
                                              name=f"pc{j}")
                        ps = st[j]
                        h = pc * HPC + j
                        for g in ((0, 1) if half == 0 else (2, 3)):
                            mm(ps, V[:, 2 * g:2 * g + 2, h, :],
                               E[:, 2 * g:2 * g + 2, j, :],
                               start=(g == 0), stop=(g == 3), perf_mode=DR)
                        if half == 1:
                            # evacuate ctx+r to SBUF (bf16) at once: the psum
                            # bank frees after ~0.3us instead of after the
                            # whole normalize round-trip, so successive tails
                            # don't serialize on the 2-bank ps_c ring.
                            cs = ctxpool.tile([P, FS], BF, tag="cs",
                                              name=f"cs{j}")
                            nc.vector.tensor_copy(cs, st.pop(j))
                            st[f"c{j}"] = cs
                    return f

                def norm():
                    ca, cb = st["c0"], st["c1"]
                    if tail_mode in ("ctx", "ctxr"):
                        nc.vector.tensor_copy(CT[0:64, pc, cols], ca[0:64, :])
                        nc.vector.tensor_copy(CT[64:P, pc, cols], cb[64:P, :])
                        return
                    # K=1 broadcast matmuls of the r rows into the dedicated
                    # ps_s bank, reciprocal, multiply.  (gpsimd
                    # partition_broadcast and DMA bounces via DRAM both give
                    # wrong results on HW -- the PE broadcast is the only
                    # correct cross-partition path found.)
                    psb = ps_s.tile([P, FS], F32, tag="pss", name="psb")
                    mm(psb[0:HD, :], ones_bf[96:97, 0:HD], ca[96:97, :],
                       tile_position=(96, 0), skip_group_check=True)
                    mm(psb[HD:P, :], ones_bf[32:33, 0:HD], cb[32:33, :],
                       tile_position=(32, HD), skip_group_check=True)
                    rb = rbpool.tile([P, FS], F32, tag="rb", name="rb")
                    nc.vector.reciprocal_approx_fast(out=rb, in_=psb)
                    nc.vector.tensor_mul(CT[0:HD, pc, cols], ca[0:HD, :],
                                         rb[0:HD, :])
                    nc.vector.tensor_mul(CT[HD:P, pc, cols], cb[HD:P, :],
                                         rb[HD:P, :])

                return [chain_piece(0, 0), chain_piece(0, 1),
                        chain_piece(1, 0), chain_piece(1, 1), norm]

            def attn_tail(qt, pc, E):
                for u in make_tail_units(qt, pc, E):
                    u()

            def attn_iter(qt, pc, fillers):
                E = attn_scores(qt, pc, fillers)
                attn_tail(qt, pc, E)

            # ---------- schedule ----------
            if ablate is not None:
                if "attn" not in ablate and "out" not in ablate:
                    nc.vector.memset(CT[:, 0, 0:16], 0.0)
                # partial builds for phase-cost ablation (timing only)
                if "qtkt" in ablate:
                    for c in range(OC):
                        for w in (1, 0):
                            qtkt_chunk(w, c)
                if "v" in ablate:
                    for sc in range(SC):
                        v_chunk(sc)
                if "sconly" in ablate:
                    for qt in range(NTS):
                        for pc in range(NPAIR):
                            attn_scores(qt, pc, [])
                elif "attnpipe" in ablate:
                    pend = None
                    for qt in range(NTS):
                        for pc in range(NPAIR):
                            E = attn_scores(qt, pc, [])
                            if pend is not None:
                                attn_tail(*pend)
                            pend = (qt, pc, E)
                    attn_tail(*pend)
                elif "attn" in ablate:
                    for qt in range(NTS):
                        for pc in range(NPAIR):
                            attn_iter(qt, pc, [])
                if "out" in ablate:
                    for sc in range(SC):
                        out_block(sc)
                return

            # prologue: QT/KT for pair 0
            qtkt_chunk(1, 0)
            qtkt_chunk(0, 0)

            def halves(f, *args):
                # two filler units per chunk; each unit emits one nt-half
                # (pieces 2p, 2p+1 adjacent: the ps_m slot is held between
                # them, so a unit is atomic wrt other ps_m chunks)
                def unit(p):
                    f(*args, piece=2 * p)
                    f(*args, piece=2 * p + 1)
                return [(lambda pp=pp: unit(pp)) for pp in range(2)]

            fill = {}
            for pc in range(NPAIR):
                fill[(0, pc)] = []
                fill[(1, pc)] = []
            # K(c) + Q(c) during iter (0, c-1).  (Moving the Q-nt1 halves to
            # their qt=1 consumer windows deadlocks the schedule -- the
            # extra ps_m users in qt1 windows close a cross-FIFO cycle with
            # the tail-unit ctx evacuations.)
            for c in range(1, OC):
                fill[(0, c - 1)] += (halves(qtkt_chunk, 1, c)
                                     + halves(qtkt_chunk, 0, c))
            # V nt0 (heads 0-7) must precede tail(0,0)'s units, interleaved
            # into window (0,2); V nt1 (heads 8-15) precedes tail(0,4) in
            # window (0,6).
            vh = {sc: halves(v_chunk, sc) for sc in range(SC)}
            for w, scs in ((0, (0, 1, 2, 3)), (1, (4, 5, 6, 7))):
                fill[(0, w)] += [vh[sc][0] for sc in scs]
            for w, scs in ((2, (0, 1)), (3, (2, 3)), (4, (4, 5)), (5, (6, 7))):
                fill[(0, w)] += [vh[sc][1] for sc in scs]
            # phase-3 for sc 0-3 (CT qt0 columns complete with tail (0,7)'s
            # units inside window (1,1) at pipeline depth 2)
            for i, sc in enumerate(range(4)):
                fill[(1, 2 + i)] += (halves(out_block, sc)
                                     + [lambda sc=sc: out_block(sc, piece=4)])

            # pipelined: tail units of iteration i interleave into the score
            # window of iteration i+2
            pending = []
            for qt in range(NTS):
                for pc in range(NPAIR):
                    tu = pending.pop(0) if len(pending) >= 2 else []
                    E = attn_scores(qt, pc, tu + fill[(qt, pc)])
                    pending.append(make_tail_units(qt, pc, E))
            for units in pending:
                for u in units:
                    u()

            # epilogue: remaining output blocks, alternating between the
            # ps_m and (now idle) ps_s banks so consecutive half-chunks
            # pipeline instead of serializing on one bank's evac round-trip
            for sc in range(4, SC):
                out_block(sc, pool=(ps_s if sc % 2 == 0 else ps_m))

            if debug_taps:
                nc.sync.dma_start(qt_o, QT.rearrange("p a b -> p (a b)"))
                nc.sync.dma_start(kt_o, KT.rearrange("p a b -> p (a b)"))
                nc.sync.dma_start(v_o.rearrange("p (a b c) -> p a b c", a=SC, b=NH), V)
                nc.sync.dma_start(ct_o, CT.rearrange("p a b -> p (a b)"))

        if loop_n == 1:
            phases()
        else:
            with tc.For_i(0, loop_n, 1) as _i:
                phases()
    nc.compile()
    return nc


def make_in_maps(input_tensor, attention_mask, Wq, bq, Wk, bk, Wv, bv, Wo, bo,
                 ln_gamma, ln_beta, S=S, H=H, NH=NH):
    SC = S // P
    OC = H // P
    X = np.asarray(input_tensor, np.float32)
    mask = np.asarray(attention_mask, np.float32)
    Wq = np.asarray(Wq, np.float32)
    Wk = np.asarray(Wk, np.float32)
    Wv = np.asarray(Wv, np.float32)
    Wo = np.asarray(Wo, np.float32)
    bq = np.asarray(bq, np.float32)
    bk = np.asarray(bk, np.float32)
    bv = np.asarray(bv, np.float32)
    bo = np.asarray(bo, np.float32)
    HD = H // NH
    sc_inv = 1.0 / np.sqrt(np.float32(HD))

    bqk = np.concatenate([
        (bq * sc_inv).reshape(OC, P).T,
        bk.reshape(OC, P).T,
    ], axis=1).astype(np.float32)  # [P, 2*OC]

    shared = {
        "wq": np.ascontiguousarray((Wq * sc_inv).T).astype(F8NP),
        "wk": np.ascontiguousarray(Wk.T).astype(F8NP),
        "wv": np.ascontiguousarray(Wv.T).astype(F8NP),
        "wo": np.ascontiguousarray(Wo.T).astype(F8NP),
        "bqk": np.ascontiguousarray(bqk),
        "bvr": bv[None, :].astype(np.float16),
        "gam": np.asarray(ln_gamma, np.float32)[None, :].astype(np.float16),
        "bet": np.asarray(ln_beta, np.float32)[None, :].astype(np.float16),
    }
    in_maps = []
    for b in range(X.shape[0]):
        m = dict(shared)
        m["xt"] = np.ascontiguousarray(X[b].T).astype(F8NP)
        m["xb"] = np.ascontiguousarray(X[b] + bo[None, :]).astype(np.float16)
        m["msk"] = np.ascontiguousarray(mask[b].reshape(SC, P).T.astype(np.float32))
        in_maps.append(m)
    return in_maps


_CACHE = {}


def _get_nc():
    if "nc" not in _CACHE:
        _CACHE["nc"] = build_program()
    return _CACHE["nc"]


def run_on_hw(in_maps, trace=False, **kw):
    from concourse.bass_utils import run_bass_kernel_spmd

    nc = _get_nc()
    return run_bass_kernel_spmd(nc, in_maps, list(range(N_CORES)), trace=trace, **kw)


def kernel(input_tensor, attention_mask, Wq, bq, Wk, bk, Wv, bv, Wo, bo,
           ln_gamma, ln_beta):
    in_maps = make_in_maps(input_tensor, attention_mask, Wq, bq, Wk, bk, Wv, bv,
                           Wo, bo, ln_gamma, ln_beta)
    res = run_on_hw(in_maps)
    return np.stack([res.results[b]["out"] for b in range(N_CORES)]).astype(np.float32)

